# revision 1
# baseline (speedup 1.0000x reference)
"""ChebNet (3x ChebConv(S=7) + FC) forward on 8 Trainium2 NeuronCores.

Strategy (graph/data parallel, per sharding hint):
- Nodes sharded 8 ways by destination; edges live with their destination.
- Per SpMV step: every core gathers source feature rows from a replicated
  node-feature table in its DRAM (dma_gather, 256B rows), segment-reduces
  per destination tile on DVE, applies the Chebyshev recurrence, and the
  8 shards are re-replicated with an AllGather for the next step.
- The symmetric-norm edge weight w = -(2/lam)*dis[row]*dis[col] is
  separable: tables store dis-prescaled features, destinations apply
  -(4/lam)*dis after the reduce, so no per-edge multiply exists.
- dma_gather indices are int16, so the ~50k-row table is read in two
  passes (A: rows [0,32768), B: rows [TABLE-32768, TABLE)); edges in the
  overlap are assigned to balance per-destination A/B counts, and
  destinations are (A,B)-sorted so per-tile padded widths stay tight.
- Layer matmuls run on TensorE with PE-transposed tiles, accumulating
  all 7 Chebyshev terms in PSUM.

Host does index preprocessing only (degrees, permutations, padded gather
slot tables); all feature compute runs on device.
"""

import math
import numpy as np

P = 128          # partitions / tile height
F = 64           # feature width (layer 1 padded 16 -> 64)
S = 7            # Chebyshev order
IDX_MAX = 32768  # int16 gather index range
NCORES = 8


class Plan:
    pass


def _balanced_pass_labels(row, src_row, deg, n, b_base):
    forcedA = src_row < b_base
    forcedB = src_row >= IDX_MAX
    flex = ~forcedA & ~forcedB
    nAf = np.bincount(row[forcedA], minlength=n)
    nfl = np.bincount(row[flex], minlength=n)
    x = np.clip((deg + 1) // 2 - nAf, 0, nfl)
    fi = np.flatnonzero(flex)
    fi = fi[np.argsort(row[fi], kind="stable")]
    r = row[fi]
    if len(r):
        first = np.r_[True, r[1:] != r[:-1]]
        gstart = np.flatnonzero(first)
        glen = np.diff(np.r_[gstart, len(r)])
        gidx = np.arange(len(r)) - np.repeat(gstart, glen)
        isa = forcedA.copy()
        isa[fi[gidx < x[r]]] = True
        return isa
    return forcedA


def build_plan(row, col, n, ncores=NCORES, w_cap=96):
    """Static gather/layout plan, structurally uniform across cores."""
    pl = Plan()
    per_core = n // ncores
    assert per_core * ncores == n
    ntiles = math.ceil(per_core / P)
    slots = ntiles * P
    shard_rows = slots + 2
    table_rows = ncores * shard_rows
    b_base = max(0, table_rows - IDX_MAX)
    assert b_base <= IDX_MAX, "table too large for two int16 gather passes"
    a_pad = slots                                  # core 0 zero row
    b_pad = (ncores - 1) * shard_rows + slots      # last core zero row

    row = np.asarray(row, dtype=np.int64)
    col = np.asarray(col, dtype=np.int64)
    deg = np.bincount(row, minlength=n).astype(np.int64)

    table_row_of_node = np.empty(n, dtype=np.int64)
    perms = [None] * ncores
    i_ar = np.arange(per_core)
    local_r = (i_ar % P) * ntiles + (i_ar // P)
    for c in range(ncores):
        nodes = np.arange(c * per_core, (c + 1) * per_core)
        perms[c] = nodes[np.argsort(-deg[nodes], kind="stable")]
        table_row_of_node[perms[c]] = c * shard_rows + local_r

    for _ in range(2):
        src_row = table_row_of_node[col]
        isa = _balanced_pass_labels(row, src_row, deg, n, b_base)
        nA = np.bincount(row[isa], minlength=n)
        nB = deg - nA
        for c in range(ncores):
            nodes = np.arange(c * per_core, (c + 1) * per_core)
            perms[c] = nodes[np.lexsort((-nB[nodes], -nA[nodes]))]
            table_row_of_node[perms[c]] = c * shard_rows + local_r
    src_row = table_row_of_node[col]
    isa = _balanced_pass_labels(row, src_row, deg, n, b_base)

    ecore = row // per_core
    lr_all = table_row_of_node[row] - ecore * shard_rows
    lt_all = lr_all % ntiles
    lp_all = lr_all // ntiles
    cntsA = np.zeros((ncores, ntiles, P), dtype=np.int64)
    cntsB = np.zeros((ncores, ntiles, P), dtype=np.int64)
    np.add.at(cntsA, (ecore[isa], lt_all[isa], lp_all[isa]), 1)
    np.add.at(cntsB, (ecore[~isa], lt_all[~isa], lp_all[~isa]), 1)
    DA = cntsA.max(axis=(0, 2))   # [ntiles], uniform over cores
    DB = cntsB.max(axis=(0, 2))
    cumA = np.r_[0, np.cumsum(DA)]
    cumB = np.r_[0, np.cumsum(DB)]

    groups = []
    t0 = 0
    while t0 < ntiles:
        t1 = t0
        wa = wb = 0
        while t1 < ntiles and (wa + DA[t1]) + (wb + DB[t1]) <= w_cap:
            wa += DA[t1]; wb += DB[t1]; t1 += 1
        assert t1 > t0, f"tile {t0}: {DA[t0]}+{DB[t0]} > w_cap"
        groups.append((t0, t1, int(wa), int(wb)))
        t0 = t1

    idx_flatA = np.full((ncores, int(DA.sum()) * P), a_pad, dtype=np.int64)
    idx_flatB = np.full((ncores, int(DB.sum()) * P), b_pad - b_base, dtype=np.int64)
    for c in range(ncores):
        m = ecore == c
        for sel, base, flat, cum in (
            (isa[m], 0, idx_flatA[c], cumA),
            (~isa[m], b_base, idx_flatB[c], cumB),
        ):
            tt, pp, ss = lt_all[m][sel], lp_all[m][sel], src_row[m][sel] - base
            if not len(tt):
                continue
            order = np.lexsort((pp, tt))
            tt, pp, ss = tt[order], pp[order], ss[order]
            key = tt * P + pp
            first = np.r_[True, key[1:] != key[:-1]]
            gstart = np.flatnonzero(first)
            glen = np.diff(np.r_[gstart, len(key)])
            gidx = np.arange(len(key)) - np.repeat(gstart, glen)
            flat[(cum[tt] + gidx) * P + pp] = ss

    def pack16(flat_idx):
        m = len(flat_idx)
        a = flat_idx.reshape(m // 16, 16).T
        assert 0 <= a.min() and a.max() <= 32767
        return np.tile(a.astype(np.int16), (8, 1))

    calls = []
    col_off = 0
    packs = [[] for _ in range(ncores)]
    for (t0, t1, WA, WB) in groups:
        for pass_, W, cum, flats in (("A", WA, cumA, idx_flatA), ("B", WB, cumB, idx_flatB)):
            if W == 0:
                calls.append(dict(pass_=pass_, t0=t0, t1=t1, W=0, col_off=0, ncols=0))
                continue
            ncols = W * P // 16
            for c in range(ncores):
                packs[c].append(pack16(flats[c][cum[t0] * P: cum[t1] * P]))
            calls.append(dict(pass_=pass_, t0=t0, t1=t1, W=int(W),
                              num_idxs=int(W) * P, col_off=col_off, ncols=ncols))
            col_off += ncols
    idx_img = np.stack([
        np.concatenate(pk, axis=1) if pk else np.zeros((P, 16), np.int16)
        for pk in packs
    ])

    pl.n, pl.ncores, pl.per_core = n, ncores, per_core
    pl.ntiles, pl.slots, pl.shard_rows, pl.table_rows = ntiles, slots, shard_rows, table_rows
    pl.b_base = b_base
    pl.deg, pl.perms = deg, perms
    pl.DA, pl.DB, pl.cumA, pl.cumB = DA, DB, cumA, cumB
    pl.groups, pl.calls = groups, calls
    pl.idx_img = idx_img
    return pl


# ----------------------------------------------------------------------
# device program
# ----------------------------------------------------------------------
def build_bass(pl, lam, n_layers=3):
    import concourse.mybir as mybir
    import concourse.bacc as bacc
    import concourse.tile as tile
    from concourse.masks import make_identity

    fp32 = mybir.dt.float32
    NT = pl.ntiles
    NTF = NT * F
    ACC1 = min(48, NT)            # tiles in the 6-bank PSUM accumulator
    IDXCOLS = pl.idx_img.shape[2]
    c1 = 2.0 / lam - 1.0
    c2 = 2.0 * c1

    nc = bacc.Bacc("TRN2", target_bir_lowering=False, debug=False,
                   num_devices=pl.ncores, dynamic_dma_scratch_size=32768)

    table0_d = nc.dram_tensor("table0", [pl.table_rows, F], fp32, kind="ExternalInput")
    t0shard_d = nc.dram_tensor("t0shard", [P, NTF], fp32, kind="ExternalInput")
    idx_d = nc.dram_tensor("idx", [P, IDXCOLS], mybir.dt.int16, kind="ExternalInput")
    av2_d = nc.dram_tensor("av2exp", [P, NTF], fp32, kind="ExternalInput")
    dis_d = nc.dram_tensor("disexp", [P, NTF], fp32, kind="ExternalInput")
    w_d = nc.dram_tensor("wmat", [n_layers, S, F, F], fp32, kind="ExternalInput")
    bias_d = nc.dram_tensor("biasb", [P, n_layers * F], fp32, kind="ExternalInput")
    wfc_d = nc.dram_tensor("wfc", [F, 1], fp32, kind="ExternalInput")
    bfc_d = nc.dram_tensor("bfc", [P, 1], fp32, kind="ExternalInput")
    out_d = nc.dram_tensor("out", [P, NT], fp32, kind="ExternalOutput")

    with tile.TileContext(nc) as tc:
        with (
            tc.tile_pool(name="const", bufs=1) as constp,
            tc.tile_pool(name="Ts", bufs=1) as tsp,
            tc.tile_pool(name="gath", bufs=2) as gp,
            tc.tile_pool(name="work", bufs=1) as wp,
            tc.tile_pool(name="small", bufs=3) as sp,
            tc.tile_pool(name="psA", bufs=1, space="PSUM") as ppa,
            tc.tile_pool(name="psT", bufs=1, space="PSUM") as ppt,
            tc.tile_pool(name="dram", bufs=2, space="DRAM") as dp,
        ):
            # ---- resident constants ----
            idx_t = constp.tile([P, IDXCOLS], mybir.dt.int16)
            nc.sync.dma_start(out=idx_t[:], in_=idx_d[:, :])
            av2_t = constp.tile([P, NTF], fp32)
            nc.sync.dma_start(out=av2_t[:], in_=av2_d[:, :])
            dis_t = constp.tile([P, NTF], fp32)
            nc.sync.dma_start(out=dis_t[:], in_=dis_d[:, :])
            w_t = constp.tile([F, n_layers * S * F], fp32)
            nc.sync.dma_start(
                out=w_t[:].rearrange("f (l s o) -> f l s o", l=n_layers, s=S),
                in_=w_d[:, :, :, :].rearrange("l s f o -> f l s o"),
            )
            bias_t = constp.tile([P, n_layers * F], fp32)
            nc.sync.dma_start(out=bias_t[:], in_=bias_d[:, :])
            wfc_t = constp.tile([F, 1], fp32)
            nc.sync.dma_start(out=wfc_t[:], in_=wfc_d[:, :])
            bfc_t = constp.tile([P, 1], fp32)
            nc.sync.dma_start(out=bfc_t[:], in_=bfc_d[:, :])
            ident_t = constp.tile([P, P], fp32)
            make_identity(nc, ident_t[:])
            zrow_t = constp.tile([2, F], fp32)
            nc.vector.memset(zrow_t[:], 0.0)

            T_bufs = [tsp.tile([P, NTF], fp32, tag=f"T{i}", name=f"Tbuf{i}")
                      for i in range(3)]
            S_t = wp.tile([P, NTF], fp32, tag="S")
            SB_t = wp.tile([P, NTF], fp32, tag="SB")

            nc.sync.dma_start(out=T_bufs[0][:], in_=t0shard_d[:, :])

            def matmul_terms(src_t, l, k, acc_sb):
                mm = ppa.tile([P, NTF], fp32, tag="mm")
                for t in range(NT):
                    tp = ppt.tile([F, P], fp32, tag="tp")
                    nc.tensor.transpose(
                        out=tp[:], in_=src_t[:, t * F:(t + 1) * F],
                        identity=ident_t[:],
                    )
                    lhsT = sp.tile([F, P], fp32, tag="lhsT")
                    nc.vector.tensor_copy(out=lhsT[:], in_=tp[:])
                    nc.tensor.matmul(
                        out=mm[:, t * F:(t + 1) * F],
                        lhsT=lhsT[:],
                        rhs=w_t[:, (l * S + k) * F:(l * S + k + 1) * F],
                        start=True,
                        stop=True,
                    )
                if k == 0:
                    nc.vector.tensor_copy(out=acc_sb[:], in_=mm[:])
                else:
                    nc.vector.tensor_tensor(
                        out=acc_sb[:], in0=acc_sb[:], in1=mm[:],
                        op=mybir.AluOpType.add,
                    )

            def spmv_gather_reduce(table_ap):
                tabA = table_ap[0:min(IDX_MAX, pl.table_rows), :]
                tabB = table_ap[pl.b_base:pl.table_rows, :]
                for gi, (t0, t1, WA, WB) in enumerate(pl.groups):
                    callA = pl.calls[2 * gi]
                    callB = pl.calls[2 * gi + 1]
                    g_t = gp.tile([P, (WA + WB) * F], fp32, tag="G")
                    for call, tab, woff, cum, DD, dst in (
                        (callA, tabA, 0, pl.cumA, pl.DA, S_t),
                        (callB, tabB, WA, pl.cumB, pl.DB, SB_t),
                    ):
                        # SWDGE carveout limit: <=1024 descriptors per call
                        for s0 in range(0, call["W"], 8):
                            w = min(8, call["W"] - s0)
                            nc.gpsimd.dma_gather(
                                g_t[:, (woff + s0) * F:(woff + s0 + w) * F].rearrange(
                                    "p (w f) -> p w f", f=F
                                ),
                                tab,
                                idx_t[:, call["col_off"] + 8 * s0:
                                      call["col_off"] + 8 * (s0 + w)],
                                w * P,
                                w * P,
                                F,
                            )
                        for t in range(t0, t1):
                            D = int(DD[t])
                            if D == 0:
                                nc.vector.memset(dst[:, t * F:(t + 1) * F], 0.0)
                                continue
                            off = woff + int(cum[t] - cum[t0])
                            gv = g_t[:, off * F:(off + D) * F].rearrange(
                                "p (d f) -> p f d", f=F
                            )
                            nc.vector.tensor_reduce(
                                out=dst[:, t * F:(t + 1) * F], in_=gv,
                                axis=mybir.AxisListType.X,
                                op=mybir.AluOpType.add,
                            )
                nc.vector.tensor_tensor(
                    out=S_t[:], in0=S_t[:], in1=SB_t[:], op=mybir.AluOpType.add
                )

            def stage_table(src_t):
                """table <- AllGather(dis * src)."""
                nc.vector.tensor_tensor(
                    out=SB_t[:], in0=src_t[:], in1=dis_t[:], op=mybir.AluOpType.mult
                )
                bounce = dp.tile([pl.shard_rows, F], fp32, tag="bounce")
                nc.sync.dma_start(
                    out=bounce[0:pl.slots, :].rearrange("(p r) f -> p r f", p=P),
                    in_=SB_t[:].rearrange("p (r f) -> p r f", f=F),
                )
                nc.sync.dma_start(out=bounce[pl.slots:pl.shard_rows, :], in_=zrow_t[:])
                table = dp.tile([pl.table_rows, F], fp32, tag="table")
                nc.gpsimd.collective_compute(
                    "AllGather",
                    mybir.AluOpType.bypass,
                    replica_groups=[list(range(pl.ncores))],
                    ins=[bounce[:, :].opt()],
                    outs=[table[:, :].opt()],
                )
                return table

            acc_sb = wp.tile([P, NTF], fp32, tag="accsb")
            cur = 0
            for l in range(n_layers):
                table = table0_d.ap() if l == 0 else stage_table(T_bufs[cur])
                matmul_terms(T_bufs[cur], l, 0, acc_sb)
                tm2 = tm1 = cur
                for k in range(1, S):
                    spmv_gather_reduce(table)
                    nc.vector.tensor_tensor(
                        out=S_t[:], in0=S_t[:], in1=av2_t[:], op=mybir.AluOpType.mult
                    )
                    if k == 1:
                        new = (cur + 1) % 3
                        nc.vector.tensor_scalar(
                            out=S_t[:], in0=S_t[:], scalar1=0.5, scalar2=None,
                            op0=mybir.AluOpType.mult,
                        )
                        nc.vector.tensor_scalar(
                            out=SB_t[:], in0=T_bufs[cur][:], scalar1=c1, scalar2=None,
                            op0=mybir.AluOpType.mult,
                        )
                        nc.vector.tensor_tensor(
                            out=T_bufs[new][:], in0=S_t[:], in1=SB_t[:],
                            op=mybir.AluOpType.add,
                        )
                        tm2, tm1 = cur, new
                    else:
                        new = 3 - tm1 - tm2
                        nc.vector.tensor_scalar(
                            out=SB_t[:], in0=T_bufs[tm1][:], scalar1=c2, scalar2=None,
                            op0=mybir.AluOpType.mult,
                        )
                        nc.vector.tensor_tensor(
                            out=S_t[:], in0=S_t[:], in1=SB_t[:],
                            op=mybir.AluOpType.add,
                        )
                        nc.vector.tensor_tensor(
                            out=T_bufs[new][:], in0=S_t[:], in1=T_bufs[tm2][:],
                            op=mybir.AluOpType.subtract,
                        )
                        tm2, tm1 = tm1, new
                    if k < S - 1:
                        table = stage_table(T_bufs[tm1])
                    matmul_terms(T_bufs[tm1], l, k, acc_sb)
                # layer output: relu(acc + bias) -> free T buffer
                outb = 3 - tm1 - tm2
                for t in range(NT):
                    nc.vector.tensor_tensor(
                        out=T_bufs[outb][:, t * F:(t + 1) * F],
                        in0=acc_sb[:, t * F:(t + 1) * F],
                        in1=bias_t[:, l * F:(l + 1) * F],
                        op=mybir.AluOpType.add,
                    )
                nc.vector.tensor_scalar(
                    out=T_bufs[outb][:], in0=T_bufs[outb][:], scalar1=0.0,
                    scalar2=None, op0=mybir.AluOpType.max,
                )
                cur = outb

            # ---- final FC ----
            out_sb = wp.tile([P, NT], fp32, tag="outsb")
            for t in range(NT):
                tp = ppt.tile([F, P], fp32, tag="tp")
                nc.tensor.transpose(
                    out=tp[:], in_=T_bufs[cur][:, t * F:(t + 1) * F],
                    identity=ident_t[:],
                )
                lhsT = sp.tile([F, P], fp32, tag="lhsT")
                nc.vector.tensor_copy(out=lhsT[:], in_=tp[:])
                fc_ps = ppt.tile([P, 1], fp32, tag="tp")
                nc.tensor.matmul(
                    out=fc_ps[:], lhsT=lhsT[:], rhs=wfc_t[:], start=True, stop=True
                )
                nc.vector.tensor_tensor(
                    out=out_sb[:, t:t + 1], in0=fc_ps[:], in1=bfc_t[:],
                    op=mybir.AluOpType.add,
                )
            nc.sync.dma_start(out=out_d[:, :], in_=out_sb[:])

    nc.compile()
    return nc


# ----------------------------------------------------------------------
# entry point
# ----------------------------------------------------------------------
def _prepare(inputs):
    x = np.asarray(inputs["x"], dtype=np.float32)
    edge_index = np.asarray(inputs["edge_index"])
    lam = float(np.asarray(inputs["lambda_max"]).reshape(-1)[0])
    n, f_in = x.shape
    row = edge_index[0].astype(np.int64)
    col = edge_index[1].astype(np.int64)

    pl = build_plan(row, col, n)
    deg = pl.deg
    dis = np.where(deg > 0, 1.0 / np.sqrt(np.maximum(deg, 1)), 0.0).astype(np.float32)

    x_pad = np.zeros((n, F), np.float32)
    x_pad[:, :f_in] = x
    W1 = np.asarray(inputs["W1"], dtype=np.float32)
    W1p = np.zeros((S, F, F), np.float32)
    W1p[:, :f_in, :] = W1
    wmat = np.stack([W1p,
                     np.asarray(inputs["W2"], np.float32),
                     np.asarray(inputs["W3"], np.float32)])
    biasb = np.zeros((P, 3 * F), np.float32)
    biasb[:, 0:F] = np.asarray(inputs["b1"], np.float32)
    biasb[:, F:2 * F] = np.asarray(inputs["b2"], np.float32)
    biasb[:, 2 * F:3 * F] = np.asarray(inputs["b3"], np.float32)
    bfc = float(np.asarray(inputs["bfc"]).reshape(-1)[0])

    NT, NTF = pl.ntiles, pl.ntiles * F
    a2 = -(4.0 / lam)
    table0 = np.zeros((pl.table_rows, F), np.float32)
    in_maps = []
    i = np.arange(pl.per_core)
    t_, p_ = i // P, i % P
    for c in range(pl.ncores):
        perm = pl.perms[c]
        sh = np.zeros((P, NT, F), np.float32)
        sh[p_, t_] = x_pad[perm]
        dl = np.zeros((P, NT), np.float32)
        dl[p_, t_] = dis[perm]
        table0[c * pl.shard_rows: c * pl.shard_rows + pl.slots] = (
            sh * dl[:, :, None]).reshape(P * NT, F)
        av2 = np.repeat((a2 * dl)[:, :, None], F, axis=2).reshape(P, NTF)
        disx = np.repeat(dl[:, :, None], F, axis=2).reshape(P, NTF)
        in_maps.append(dict(
            t0shard=np.ascontiguousarray(sh.reshape(P, NTF)),
            idx=np.ascontiguousarray(pl.idx_img[c]),
            av2exp=np.ascontiguousarray(av2.astype(np.float32)),
            disexp=np.ascontiguousarray(disx.astype(np.float32)),
            wmat=wmat,
            biasb=biasb,
            wfc=np.asarray(inputs["Wfc"], np.float32),
            bfc=np.full((P, 1), bfc, np.float32),
        ))
    for m in in_maps:
        m["table0"] = table0
    return pl, lam, in_maps


def _run(inputs, trace=False):
    from concourse.bass_utils import run_bass_kernel_spmd

    pl, lam, in_maps = _prepare(inputs)
    nc = build_bass(pl, lam)
    res = run_bass_kernel_spmd(
        nc, in_maps, core_ids=list(range(pl.ncores)), trace=trace
    )
    n = pl.n
    y = np.zeros((n, 1), np.float32)
    i = np.arange(pl.per_core)
    for c in range(pl.ncores):
        o = np.asarray(res.results[c]["out"])
        y[pl.perms[c], 0] = o[i % P, i // P]
    return y, res


def kernel(**inputs) -> np.ndarray:
    y, _ = _run(inputs, trace=False)
    return y



# revision 4
# speedup vs baseline: 1.0653x; 1.0653x over previous
"""ChebNet (3x ChebConv(S=7) + FC) forward on 8 Trainium2 NeuronCores.

Strategy (graph/data parallel, per sharding hint):
- Nodes sharded 8 ways by destination; edges live with their destination.
- Per SpMV step: every core gathers source feature rows from a replicated
  node-feature table in its DRAM (dma_gather, 256B rows), segment-reduces
  per destination tile on DVE, applies the Chebyshev recurrence, and the
  8 shards are re-replicated with an AllGather for the next step.
- The symmetric-norm edge weight w = -(2/lam)*dis[row]*dis[col] is
  separable: tables store dis-prescaled features, destinations apply
  -(4/lam)*dis after the reduce, so no per-edge multiply exists.
- dma_gather indices are int16, so the ~50k-row table is read in two
  passes (A: rows [0,32768), B: rows [TABLE-32768, TABLE)); edges in the
  overlap are assigned to balance per-destination A/B counts, and
  destinations are (A,B)-sorted so per-tile padded widths stay tight.
- Layer matmuls run on TensorE with PE-transposed tiles, accumulating
  all 7 Chebyshev terms in PSUM.

Host does index preprocessing only (degrees, permutations, padded gather
slot tables); all feature compute runs on device.
"""

import math
import numpy as np

P = 128          # partitions / tile height
F = 64           # feature width (layer 1 padded 16 -> 64)
S = 7            # Chebyshev order
IDX_MAX = 32768  # int16 gather index range
NCORES = 8


class Plan:
    pass


def _balanced_pass_labels(row, src_row, deg, n, b_base):
    forcedA = src_row < b_base
    forcedB = src_row >= IDX_MAX
    flex = ~forcedA & ~forcedB
    nAf = np.bincount(row[forcedA], minlength=n)
    nfl = np.bincount(row[flex], minlength=n)
    x = np.clip((deg + 1) // 2 - nAf, 0, nfl)
    fi = np.flatnonzero(flex)
    fi = fi[np.argsort(row[fi], kind="stable")]
    r = row[fi]
    if len(r):
        first = np.r_[True, r[1:] != r[:-1]]
        gstart = np.flatnonzero(first)
        glen = np.diff(np.r_[gstart, len(r)])
        gidx = np.arange(len(r)) - np.repeat(gstart, glen)
        isa = forcedA.copy()
        isa[fi[gidx < x[r]]] = True
        return isa
    return forcedA


def build_plan(row, col, n, ncores=NCORES, w_cap=96):
    """Static gather/layout plan, structurally uniform across cores."""
    pl = Plan()
    per_core = n // ncores
    assert per_core * ncores == n
    ntiles = math.ceil(per_core / P)
    slots = ntiles * P
    shard_rows = slots + 2
    table_rows = ncores * shard_rows
    b_base = max(0, table_rows - IDX_MAX)
    assert b_base <= IDX_MAX, "table too large for two int16 gather passes"
    a_pad = slots                                  # core 0 zero row
    b_pad = (ncores - 1) * shard_rows + slots      # last core zero row

    row = np.asarray(row, dtype=np.int64)
    col = np.asarray(col, dtype=np.int64)
    deg = np.bincount(row, minlength=n).astype(np.int64)

    table_row_of_node = np.empty(n, dtype=np.int64)
    perms = [None] * ncores
    i_ar = np.arange(per_core)
    local_r = (i_ar % P) * ntiles + (i_ar // P)
    for c in range(ncores):
        nodes = np.arange(c * per_core, (c + 1) * per_core)
        perms[c] = nodes[np.argsort(-deg[nodes], kind="stable")]
        table_row_of_node[perms[c]] = c * shard_rows + local_r

    for _ in range(4):
        src_row = table_row_of_node[col]
        isa = _balanced_pass_labels(row, src_row, deg, n, b_base)
        nA = np.bincount(row[isa], minlength=n)
        nB = deg - nA
        for c in range(ncores):
            nodes = np.arange(c * per_core, (c + 1) * per_core)
            perms[c] = nodes[np.lexsort((-nA[nodes], -nB[nodes]))]
            table_row_of_node[perms[c]] = c * shard_rows + local_r
    src_row = table_row_of_node[col]
    isa = _balanced_pass_labels(row, src_row, deg, n, b_base)

    ecore = row // per_core
    lr_all = table_row_of_node[row] - ecore * shard_rows
    lt_all = lr_all % ntiles
    lp_all = lr_all // ntiles
    cntsA = np.zeros((ncores, ntiles, P), dtype=np.int64)
    cntsB = np.zeros((ncores, ntiles, P), dtype=np.int64)
    np.add.at(cntsA, (ecore[isa], lt_all[isa], lp_all[isa]), 1)
    np.add.at(cntsB, (ecore[~isa], lt_all[~isa], lp_all[~isa]), 1)
    DA = cntsA.max(axis=(0, 2))   # [ntiles], uniform over cores
    DB = cntsB.max(axis=(0, 2))
    cumA = np.r_[0, np.cumsum(DA)]
    cumB = np.r_[0, np.cumsum(DB)]

    groups = []
    t0 = 0
    while t0 < ntiles:
        t1 = t0
        wa = wb = 0
        while t1 < ntiles and (wa + DA[t1]) + (wb + DB[t1]) <= w_cap:
            wa += DA[t1]; wb += DB[t1]; t1 += 1
        assert t1 > t0, f"tile {t0}: {DA[t0]}+{DB[t0]} > w_cap"
        groups.append((t0, t1, int(wa), int(wb)))
        t0 = t1

    idx_flatA = np.full((ncores, int(DA.sum()) * P), a_pad, dtype=np.int64)
    idx_flatB = np.full((ncores, int(DB.sum()) * P), b_pad - b_base, dtype=np.int64)
    for c in range(ncores):
        m = ecore == c
        for sel, base, flat, cum in (
            (isa[m], 0, idx_flatA[c], cumA),
            (~isa[m], b_base, idx_flatB[c], cumB),
        ):
            tt, pp, ss = lt_all[m][sel], lp_all[m][sel], src_row[m][sel] - base
            if not len(tt):
                continue
            order = np.lexsort((pp, tt))
            tt, pp, ss = tt[order], pp[order], ss[order]
            key = tt * P + pp
            first = np.r_[True, key[1:] != key[:-1]]
            gstart = np.flatnonzero(first)
            glen = np.diff(np.r_[gstart, len(key)])
            gidx = np.arange(len(key)) - np.repeat(gstart, glen)
            flat[(cum[tt] + gidx) * P + pp] = ss

    def pack16(flat_idx):
        m = len(flat_idx)
        a = flat_idx.reshape(m // 16, 16).T
        assert 0 <= a.min() and a.max() <= 32767
        return np.tile(a.astype(np.int16), (8, 1))

    calls = []
    col_off = 0
    packs = [[] for _ in range(ncores)]
    for (t0, t1, WA, WB) in groups:
        for pass_, W, cum, flats in (("A", WA, cumA, idx_flatA), ("B", WB, cumB, idx_flatB)):
            if W == 0:
                calls.append(dict(pass_=pass_, t0=t0, t1=t1, W=0, col_off=0, ncols=0))
                continue
            ncols = W * P // 16
            for c in range(ncores):
                packs[c].append(pack16(flats[c][cum[t0] * P: cum[t1] * P]))
            calls.append(dict(pass_=pass_, t0=t0, t1=t1, W=int(W),
                              num_idxs=int(W) * P, col_off=col_off, ncols=ncols))
            col_off += ncols
    idx_img = np.stack([
        np.concatenate(pk, axis=1) if pk else np.zeros((P, 16), np.int16)
        for pk in packs
    ])

    pl.n, pl.ncores, pl.per_core = n, ncores, per_core
    pl.ntiles, pl.slots, pl.shard_rows, pl.table_rows = ntiles, slots, shard_rows, table_rows
    pl.b_base = b_base
    pl.deg, pl.perms = deg, perms
    pl.DA, pl.DB, pl.cumA, pl.cumB = DA, DB, cumA, cumB
    pl.groups, pl.calls = groups, calls
    pl.idx_img = idx_img
    return pl


# ----------------------------------------------------------------------
# device program
# ----------------------------------------------------------------------
def build_bass(pl, lam, n_layers=3):
    import concourse.mybir as mybir
    import concourse.bacc as bacc
    import concourse.tile as tile
    from concourse.masks import make_identity

    fp32 = mybir.dt.float32
    NT = pl.ntiles
    NTF = NT * F
    ACC1 = min(48, NT)            # tiles in the 6-bank PSUM accumulator
    IDXCOLS = pl.idx_img.shape[2]
    c1 = 2.0 / lam - 1.0
    c2 = 2.0 * c1

    nc = bacc.Bacc("TRN2", target_bir_lowering=False, debug=False,
                   num_devices=pl.ncores, dynamic_dma_scratch_size=32768)

    table0_d = nc.dram_tensor("table0", [pl.table_rows, F], fp32, kind="ExternalInput")
    t0shard_d = nc.dram_tensor("t0shard", [P, NTF], fp32, kind="ExternalInput")
    idx_d = nc.dram_tensor("idx", [P, IDXCOLS], mybir.dt.int16, kind="ExternalInput")
    av2_d = nc.dram_tensor("av2exp", [P, NTF], fp32, kind="ExternalInput")
    dis_d = nc.dram_tensor("disexp", [P, NTF], fp32, kind="ExternalInput")
    w_d = nc.dram_tensor("wmat", [n_layers, S, F, F], fp32, kind="ExternalInput")
    bias_d = nc.dram_tensor("biasb", [P, n_layers * F], fp32, kind="ExternalInput")
    wfc_d = nc.dram_tensor("wfc", [F, 1], fp32, kind="ExternalInput")
    bfc_d = nc.dram_tensor("bfc", [P, 1], fp32, kind="ExternalInput")
    out_d = nc.dram_tensor("out", [P, NT], fp32, kind="ExternalOutput")

    with tile.TileContext(nc) as tc:
        with (
            tc.tile_pool(name="const", bufs=1) as constp,
            tc.tile_pool(name="Ts", bufs=1) as tsp,
            tc.tile_pool(name="gath", bufs=2) as gp,
            tc.tile_pool(name="work", bufs=1) as wp,
            tc.tile_pool(name="small", bufs=3) as sp,
            tc.tile_pool(name="psA", bufs=1, space="PSUM") as ppa,
            tc.tile_pool(name="psT", bufs=1, space="PSUM") as ppt,
            tc.tile_pool(name="dram", bufs=2, space="DRAM") as dp,
        ):
            # ---- resident constants ----
            idx_t = constp.tile([P, IDXCOLS], mybir.dt.int16)
            nc.sync.dma_start(out=idx_t[:], in_=idx_d[:, :])
            av2_t = constp.tile([P, NTF], fp32)
            nc.sync.dma_start(out=av2_t[:], in_=av2_d[:, :])
            dis_t = constp.tile([P, NTF], fp32)
            nc.sync.dma_start(out=dis_t[:], in_=dis_d[:, :])
            w_t = constp.tile([F, n_layers * S * F], fp32)
            nc.sync.dma_start(
                out=w_t[:].rearrange("f (l s o) -> f l s o", l=n_layers, s=S),
                in_=w_d[:, :, :, :].rearrange("l s f o -> f l s o"),
            )
            bias_t = constp.tile([P, n_layers * F], fp32)
            nc.sync.dma_start(out=bias_t[:], in_=bias_d[:, :])
            wfc_t = constp.tile([F, 1], fp32)
            nc.sync.dma_start(out=wfc_t[:], in_=wfc_d[:, :])
            bfc_t = constp.tile([P, 1], fp32)
            nc.sync.dma_start(out=bfc_t[:], in_=bfc_d[:, :])
            ident_t = constp.tile([P, P], fp32)
            make_identity(nc, ident_t[:])
            zrow_t = constp.tile([2, F], fp32)
            nc.vector.memset(zrow_t[:], 0.0)

            T_bufs = [tsp.tile([P, NTF], fp32, tag=f"T{i}", name=f"Tbuf{i}")
                      for i in range(3)]
            S_t = wp.tile([P, NTF], fp32, tag="S")
            SB_t = wp.tile([P, NTF], fp32, tag="SB")

            nc.sync.dma_start(out=T_bufs[0][:], in_=t0shard_d[:, :])

            def matmul_terms(src_t, l, k, acc_sb):
                mm = ppa.tile([P, NTF], fp32, tag="mm")
                for t in range(NT):
                    tp = ppt.tile([F, P], fp32, tag="tp")
                    nc.tensor.transpose(
                        out=tp[:], in_=src_t[:, t * F:(t + 1) * F],
                        identity=ident_t[:],
                    )
                    lhsT = sp.tile([F, P], fp32, tag="lhsT")
                    nc.vector.tensor_copy(out=lhsT[:], in_=tp[:])
                    nc.tensor.matmul(
                        out=mm[:, t * F:(t + 1) * F],
                        lhsT=lhsT[:],
                        rhs=w_t[:, (l * S + k) * F:(l * S + k + 1) * F],
                        start=True,
                        stop=True,
                    )
                if k == 0:
                    nc.vector.tensor_copy(out=acc_sb[:], in_=mm[:])
                else:
                    nc.vector.tensor_tensor(
                        out=acc_sb[:], in0=acc_sb[:], in1=mm[:],
                        op=mybir.AluOpType.add,
                    )

            def spmv_gather_reduce(table_ap):
                tabA = table_ap[0:min(IDX_MAX, pl.table_rows), :]
                tabB = table_ap[pl.b_base:pl.table_rows, :]
                for gi, (t0, t1, WA, WB) in enumerate(pl.groups):
                    callA = pl.calls[2 * gi]
                    callB = pl.calls[2 * gi + 1]
                    g_t = gp.tile([P, (WA + WB) * F], fp32, tag="G")
                    for call, tab, woff, cum, DD, dst in (
                        (callA, tabA, 0, pl.cumA, pl.DA, S_t),
                        (callB, tabB, WA, pl.cumB, pl.DB, SB_t),
                    ):
                        # SWDGE carveout limit: <=1024 descriptors per call
                        for s0 in range(0, call["W"], 8):
                            w = min(8, call["W"] - s0)
                            nc.gpsimd.dma_gather(
                                g_t[:, (woff + s0) * F:(woff + s0 + w) * F].rearrange(
                                    "p (w f) -> p w f", f=F
                                ),
                                tab,
                                idx_t[:, call["col_off"] + 8 * s0:
                                      call["col_off"] + 8 * (s0 + w)],
                                w * P,
                                w * P,
                                F,
                            )
                        for t in range(t0, t1):
                            D = int(DD[t])
                            if D == 0:
                                nc.vector.memset(dst[:, t * F:(t + 1) * F], 0.0)
                                continue
                            off = woff + int(cum[t] - cum[t0])
                            gv = g_t[:, off * F:(off + D) * F].rearrange(
                                "p (d f) -> p f d", f=F
                            )
                            nc.vector.tensor_reduce(
                                out=dst[:, t * F:(t + 1) * F], in_=gv,
                                axis=mybir.AxisListType.X,
                                op=mybir.AluOpType.add,
                            )
                nc.vector.tensor_tensor(
                    out=S_t[:], in0=S_t[:], in1=SB_t[:], op=mybir.AluOpType.add
                )

            def stage_table(src_t):
                """table <- AllGather(dis * src)."""
                nc.vector.tensor_tensor(
                    out=SB_t[:], in0=src_t[:], in1=dis_t[:], op=mybir.AluOpType.mult
                )
                bounce = dp.tile([pl.shard_rows, F], fp32, tag="bounce")
                nc.sync.dma_start(
                    out=bounce[0:pl.slots, :].rearrange("(p r) f -> p r f", p=P),
                    in_=SB_t[:].rearrange("p (r f) -> p r f", f=F),
                )
                nc.sync.dma_start(out=bounce[pl.slots:pl.shard_rows, :], in_=zrow_t[:])
                table = dp.tile([pl.table_rows, F], fp32, tag="table")
                nc.gpsimd.collective_compute(
                    "AllGather",
                    mybir.AluOpType.bypass,
                    replica_groups=[list(range(pl.ncores))],
                    ins=[bounce[:, :].opt()],
                    outs=[table[:, :].opt()],
                )
                return table

            acc_sb = wp.tile([P, NTF], fp32, tag="accsb")
            cur = 0
            for l in range(n_layers):
                table = table0_d.ap() if l == 0 else stage_table(T_bufs[cur])
                matmul_terms(T_bufs[cur], l, 0, acc_sb)
                tm2 = tm1 = cur
                for k in range(1, S):
                    spmv_gather_reduce(table)
                    nc.vector.tensor_tensor(
                        out=S_t[:], in0=S_t[:], in1=av2_t[:], op=mybir.AluOpType.mult
                    )
                    if k == 1:
                        new = (cur + 1) % 3
                        nc.vector.tensor_scalar(
                            out=S_t[:], in0=S_t[:], scalar1=0.5, scalar2=None,
                            op0=mybir.AluOpType.mult,
                        )
                        nc.vector.tensor_scalar(
                            out=SB_t[:], in0=T_bufs[cur][:], scalar1=c1, scalar2=None,
                            op0=mybir.AluOpType.mult,
                        )
                        nc.vector.tensor_tensor(
                            out=T_bufs[new][:], in0=S_t[:], in1=SB_t[:],
                            op=mybir.AluOpType.add,
                        )
                        tm2, tm1 = cur, new
                    else:
                        new = 3 - tm1 - tm2
                        nc.vector.tensor_scalar(
                            out=SB_t[:], in0=T_bufs[tm1][:], scalar1=c2, scalar2=None,
                            op0=mybir.AluOpType.mult,
                        )
                        nc.vector.tensor_tensor(
                            out=S_t[:], in0=S_t[:], in1=SB_t[:],
                            op=mybir.AluOpType.add,
                        )
                        nc.vector.tensor_tensor(
                            out=T_bufs[new][:], in0=S_t[:], in1=T_bufs[tm2][:],
                            op=mybir.AluOpType.subtract,
                        )
                        tm2, tm1 = tm1, new
                    if k < S - 1:
                        table = stage_table(T_bufs[tm1])
                    matmul_terms(T_bufs[tm1], l, k, acc_sb)
                # layer output: relu(acc + bias) -> free T buffer
                outb = 3 - tm1 - tm2
                for t in range(NT):
                    nc.vector.tensor_tensor(
                        out=T_bufs[outb][:, t * F:(t + 1) * F],
                        in0=acc_sb[:, t * F:(t + 1) * F],
                        in1=bias_t[:, l * F:(l + 1) * F],
                        op=mybir.AluOpType.add,
                    )
                nc.vector.tensor_scalar(
                    out=T_bufs[outb][:], in0=T_bufs[outb][:], scalar1=0.0,
                    scalar2=None, op0=mybir.AluOpType.max,
                )
                cur = outb

            # ---- final FC ----
            out_sb = wp.tile([P, NT], fp32, tag="outsb")
            for t in range(NT):
                tp = ppt.tile([F, P], fp32, tag="tp")
                nc.tensor.transpose(
                    out=tp[:], in_=T_bufs[cur][:, t * F:(t + 1) * F],
                    identity=ident_t[:],
                )
                lhsT = sp.tile([F, P], fp32, tag="lhsT")
                nc.vector.tensor_copy(out=lhsT[:], in_=tp[:])
                fc_ps = ppt.tile([P, 1], fp32, tag="tp")
                nc.tensor.matmul(
                    out=fc_ps[:], lhsT=lhsT[:], rhs=wfc_t[:], start=True, stop=True
                )
                nc.vector.tensor_tensor(
                    out=out_sb[:, t:t + 1], in0=fc_ps[:], in1=bfc_t[:],
                    op=mybir.AluOpType.add,
                )
            nc.sync.dma_start(out=out_d[:, :], in_=out_sb[:])

    nc.compile()
    return nc


# ----------------------------------------------------------------------
# entry point
# ----------------------------------------------------------------------
def _prepare(inputs):
    x = np.asarray(inputs["x"], dtype=np.float32)
    edge_index = np.asarray(inputs["edge_index"])
    lam = float(np.asarray(inputs["lambda_max"]).reshape(-1)[0])
    n, f_in = x.shape
    row = edge_index[0].astype(np.int64)
    col = edge_index[1].astype(np.int64)

    pl = build_plan(row, col, n)
    deg = pl.deg
    dis = np.where(deg > 0, 1.0 / np.sqrt(np.maximum(deg, 1)), 0.0).astype(np.float32)

    x_pad = np.zeros((n, F), np.float32)
    x_pad[:, :f_in] = x
    W1 = np.asarray(inputs["W1"], dtype=np.float32)
    W1p = np.zeros((S, F, F), np.float32)
    W1p[:, :f_in, :] = W1
    wmat = np.stack([W1p,
                     np.asarray(inputs["W2"], np.float32),
                     np.asarray(inputs["W3"], np.float32)])
    biasb = np.zeros((P, 3 * F), np.float32)
    biasb[:, 0:F] = np.asarray(inputs["b1"], np.float32)
    biasb[:, F:2 * F] = np.asarray(inputs["b2"], np.float32)
    biasb[:, 2 * F:3 * F] = np.asarray(inputs["b3"], np.float32)
    bfc = float(np.asarray(inputs["bfc"]).reshape(-1)[0])

    NT, NTF = pl.ntiles, pl.ntiles * F
    a2 = -(4.0 / lam)
    table0 = np.zeros((pl.table_rows, F), np.float32)
    in_maps = []
    i = np.arange(pl.per_core)
    t_, p_ = i // P, i % P
    for c in range(pl.ncores):
        perm = pl.perms[c]
        sh = np.zeros((P, NT, F), np.float32)
        sh[p_, t_] = x_pad[perm]
        dl = np.zeros((P, NT), np.float32)
        dl[p_, t_] = dis[perm]
        table0[c * pl.shard_rows: c * pl.shard_rows + pl.slots] = (
            sh * dl[:, :, None]).reshape(P * NT, F)
        av2 = np.repeat((a2 * dl)[:, :, None], F, axis=2).reshape(P, NTF)
        disx = np.repeat(dl[:, :, None], F, axis=2).reshape(P, NTF)
        in_maps.append(dict(
            t0shard=np.ascontiguousarray(sh.reshape(P, NTF)),
            idx=np.ascontiguousarray(pl.idx_img[c]),
            av2exp=np.ascontiguousarray(av2.astype(np.float32)),
            disexp=np.ascontiguousarray(disx.astype(np.float32)),
            wmat=wmat,
            biasb=biasb,
            wfc=np.asarray(inputs["Wfc"], np.float32),
            bfc=np.full((P, 1), bfc, np.float32),
        ))
    for m in in_maps:
        m["table0"] = table0
    return pl, lam, in_maps


def _run(inputs, trace=False):
    from concourse.bass_utils import run_bass_kernel_spmd

    pl, lam, in_maps = _prepare(inputs)
    nc = build_bass(pl, lam)
    res = run_bass_kernel_spmd(
        nc, in_maps, core_ids=list(range(pl.ncores)), trace=trace
    )
    n = pl.n
    y = np.zeros((n, 1), np.float32)
    i = np.arange(pl.per_core)
    for c in range(pl.ncores):
        o = np.asarray(res.results[c]["out"])
        y[pl.perms[c], 0] = o[i % P, i // P]
    return y, res


def kernel(**inputs) -> np.ndarray:
    y, _ = _run(inputs, trace=False)
    return y



# revision 5
# speedup vs baseline: 1.0751x; 1.0092x over previous
"""ChebNet (3x ChebConv(S=7) + FC) forward on 8 Trainium2 NeuronCores.

Strategy (graph/data parallel, per sharding hint):
- Nodes sharded 8 ways by destination; edges live with their destination.
- Per SpMV step: every core gathers source feature rows from a replicated
  node-feature table in its DRAM (dma_gather, 256B rows), segment-reduces
  per destination tile on DVE, applies the Chebyshev recurrence, and the
  8 shards are re-replicated with an AllGather for the next step.
- The symmetric-norm edge weight w = -(2/lam)*dis[row]*dis[col] is
  separable: tables store dis-prescaled features, destinations apply
  -(4/lam)*dis after the reduce, so no per-edge multiply exists.
- dma_gather indices are int16, so the ~50k-row table is read in two
  passes (A: rows [0,32768), B: rows [TABLE-32768, TABLE)); edges in the
  overlap are assigned to balance per-destination A/B counts, and
  destinations are (A,B)-sorted so per-tile padded widths stay tight.
- Layer matmuls run on TensorE with PE-transposed tiles, accumulating
  all 7 Chebyshev terms in PSUM.

Host does index preprocessing only (degrees, permutations, padded gather
slot tables); all feature compute runs on device.
"""

import math
import numpy as np

P = 128          # partitions / tile height
F = 64           # feature width (layer 1 padded 16 -> 64)
S = 7            # Chebyshev order
IDX_MAX = 32768  # int16 gather index range
NCORES = 8


class Plan:
    pass


def _balanced_pass_labels(row, src_row, deg, n, b_base):
    forcedA = src_row < b_base
    forcedB = src_row >= IDX_MAX
    flex = ~forcedA & ~forcedB
    nAf = np.bincount(row[forcedA], minlength=n)
    nfl = np.bincount(row[flex], minlength=n)
    x = np.clip((deg + 1) // 2 - nAf, 0, nfl)
    fi = np.flatnonzero(flex)
    fi = fi[np.argsort(row[fi], kind="stable")]
    r = row[fi]
    if len(r):
        first = np.r_[True, r[1:] != r[:-1]]
        gstart = np.flatnonzero(first)
        glen = np.diff(np.r_[gstart, len(r)])
        gidx = np.arange(len(r)) - np.repeat(gstart, glen)
        isa = forcedA.copy()
        isa[fi[gidx < x[r]]] = True
        return isa
    return forcedA


def build_plan(row, col, n, ncores=NCORES, w_cap=128):
    """Static gather/layout plan, structurally uniform across cores."""
    pl = Plan()
    per_core = n // ncores
    assert per_core * ncores == n
    ntiles = math.ceil(per_core / P)
    slots = ntiles * P
    shard_rows = slots + 2
    table_rows = ncores * shard_rows
    b_base = max(0, table_rows - IDX_MAX)
    assert b_base <= IDX_MAX, "table too large for two int16 gather passes"
    a_pad = slots                                  # core 0 zero row
    b_pad = (ncores - 1) * shard_rows + slots      # last core zero row

    row = np.asarray(row, dtype=np.int64)
    col = np.asarray(col, dtype=np.int64)
    deg = np.bincount(row, minlength=n).astype(np.int64)

    table_row_of_node = np.empty(n, dtype=np.int64)
    perms = [None] * ncores
    i_ar = np.arange(per_core)
    local_r = (i_ar % P) * ntiles + (i_ar // P)
    for c in range(ncores):
        nodes = np.arange(c * per_core, (c + 1) * per_core)
        perms[c] = nodes[np.argsort(-deg[nodes], kind="stable")]
        table_row_of_node[perms[c]] = c * shard_rows + local_r

    for _ in range(4):
        src_row = table_row_of_node[col]
        isa = _balanced_pass_labels(row, src_row, deg, n, b_base)
        nA = np.bincount(row[isa], minlength=n)
        nB = deg - nA
        for c in range(ncores):
            nodes = np.arange(c * per_core, (c + 1) * per_core)
            perms[c] = nodes[np.lexsort((-nA[nodes], -nB[nodes]))]
            table_row_of_node[perms[c]] = c * shard_rows + local_r
    src_row = table_row_of_node[col]
    isa = _balanced_pass_labels(row, src_row, deg, n, b_base)

    ecore = row // per_core
    lr_all = table_row_of_node[row] - ecore * shard_rows
    lt_all = lr_all % ntiles
    lp_all = lr_all // ntiles
    cntsA = np.zeros((ncores, ntiles, P), dtype=np.int64)
    cntsB = np.zeros((ncores, ntiles, P), dtype=np.int64)
    np.add.at(cntsA, (ecore[isa], lt_all[isa], lp_all[isa]), 1)
    np.add.at(cntsB, (ecore[~isa], lt_all[~isa], lp_all[~isa]), 1)
    DA = cntsA.max(axis=(0, 2))   # [ntiles], uniform over cores
    DB = cntsB.max(axis=(0, 2))
    cumA = np.r_[0, np.cumsum(DA)]
    cumB = np.r_[0, np.cumsum(DB)]

    groups = []
    t0 = 0
    while t0 < ntiles:
        t1 = t0
        wa = wb = 0
        while t1 < ntiles and (wa + DA[t1]) + (wb + DB[t1]) <= w_cap:
            wa += DA[t1]; wb += DB[t1]; t1 += 1
        assert t1 > t0, f"tile {t0}: {DA[t0]}+{DB[t0]} > w_cap"
        groups.append((t0, t1, int(wa), int(wb)))
        t0 = t1

    idx_flatA = np.full((ncores, int(DA.sum()) * P), a_pad, dtype=np.int64)
    idx_flatB = np.full((ncores, int(DB.sum()) * P), b_pad - b_base, dtype=np.int64)
    for c in range(ncores):
        m = ecore == c
        for sel, base, flat, cum in (
            (isa[m], 0, idx_flatA[c], cumA),
            (~isa[m], b_base, idx_flatB[c], cumB),
        ):
            tt, pp, ss = lt_all[m][sel], lp_all[m][sel], src_row[m][sel] - base
            if not len(tt):
                continue
            order = np.lexsort((pp, tt))
            tt, pp, ss = tt[order], pp[order], ss[order]
            key = tt * P + pp
            first = np.r_[True, key[1:] != key[:-1]]
            gstart = np.flatnonzero(first)
            glen = np.diff(np.r_[gstart, len(key)])
            gidx = np.arange(len(key)) - np.repeat(gstart, glen)
            flat[(cum[tt] + gidx) * P + pp] = ss

    def pack16(flat_idx):
        m = len(flat_idx)
        a = flat_idx.reshape(m // 16, 16).T
        assert 0 <= a.min() and a.max() <= 32767
        return np.tile(a.astype(np.int16), (8, 1))

    calls = []
    col_off = 0
    packs = [[] for _ in range(ncores)]
    for (t0, t1, WA, WB) in groups:
        for pass_, W, cum, flats in (("A", WA, cumA, idx_flatA), ("B", WB, cumB, idx_flatB)):
            if W == 0:
                calls.append(dict(pass_=pass_, t0=t0, t1=t1, W=0, col_off=0, ncols=0))
                continue
            ncols = W * P // 16
            for c in range(ncores):
                packs[c].append(pack16(flats[c][cum[t0] * P: cum[t1] * P]))
            calls.append(dict(pass_=pass_, t0=t0, t1=t1, W=int(W),
                              num_idxs=int(W) * P, col_off=col_off, ncols=ncols))
            col_off += ncols
    idx_img = np.stack([
        np.concatenate(pk, axis=1) if pk else np.zeros((P, 16), np.int16)
        for pk in packs
    ])

    pl.n, pl.ncores, pl.per_core = n, ncores, per_core
    pl.ntiles, pl.slots, pl.shard_rows, pl.table_rows = ntiles, slots, shard_rows, table_rows
    pl.b_base = b_base
    pl.deg, pl.perms = deg, perms
    pl.DA, pl.DB, pl.cumA, pl.cumB = DA, DB, cumA, cumB
    pl.groups, pl.calls = groups, calls
    pl.idx_img = idx_img
    return pl


# ----------------------------------------------------------------------
# device program
# ----------------------------------------------------------------------
def build_bass(pl, lam, n_layers=3):
    import concourse.mybir as mybir
    import concourse.bacc as bacc
    import concourse.tile as tile
    from concourse.masks import make_identity

    fp32 = mybir.dt.float32
    NT = pl.ntiles
    NTF = NT * F
    ACC1 = min(48, NT)            # tiles in the 6-bank PSUM accumulator
    IDXCOLS = pl.idx_img.shape[2]
    c1 = 2.0 / lam - 1.0
    c2 = 2.0 * c1

    nc = bacc.Bacc("TRN2", target_bir_lowering=False, debug=False,
                   num_devices=pl.ncores, dynamic_dma_scratch_size=32768)

    table0_d = nc.dram_tensor("table0", [pl.table_rows, F], fp32, kind="ExternalInput")
    t0shard_d = nc.dram_tensor("t0shard", [P, NTF], fp32, kind="ExternalInput")
    idx_d = nc.dram_tensor("idx", [P, IDXCOLS], mybir.dt.int16, kind="ExternalInput")
    av2_d = nc.dram_tensor("av2exp", [P, NTF], fp32, kind="ExternalInput")
    dis_d = nc.dram_tensor("disexp", [P, NTF], fp32, kind="ExternalInput")
    w_d = nc.dram_tensor("wmat", [n_layers, S, F, F], fp32, kind="ExternalInput")
    bias_d = nc.dram_tensor("biasb", [P, n_layers * F], fp32, kind="ExternalInput")
    wfc_d = nc.dram_tensor("wfc", [F, 1], fp32, kind="ExternalInput")
    bfc_d = nc.dram_tensor("bfc", [P, 1], fp32, kind="ExternalInput")
    out_d = nc.dram_tensor("out", [P, NT], fp32, kind="ExternalOutput")

    with tile.TileContext(nc) as tc:
        with (
            tc.tile_pool(name="const", bufs=1) as constp,
            tc.tile_pool(name="Ts", bufs=1) as tsp,
            tc.tile_pool(name="gath", bufs=2) as gp,
            tc.tile_pool(name="work", bufs=1) as wp,
            tc.tile_pool(name="small", bufs=3) as sp,
            tc.tile_pool(name="psA", bufs=1, space="PSUM") as ppa,
            tc.tile_pool(name="psT", bufs=1, space="PSUM") as ppt,
            tc.tile_pool(name="dram", bufs=2, space="DRAM") as dp,
        ):
            # ---- resident constants ----
            idx_t = constp.tile([P, IDXCOLS], mybir.dt.int16)
            nc.sync.dma_start(out=idx_t[:], in_=idx_d[:, :])
            av2_t = constp.tile([P, NTF], fp32)
            nc.sync.dma_start(out=av2_t[:], in_=av2_d[:, :])
            dis_t = constp.tile([P, NTF], fp32)
            nc.sync.dma_start(out=dis_t[:], in_=dis_d[:, :])
            w_t = constp.tile([F, n_layers * S * F], fp32)
            nc.sync.dma_start(
                out=w_t[:].rearrange("f (l s o) -> f l s o", l=n_layers, s=S),
                in_=w_d[:, :, :, :].rearrange("l s f o -> f l s o"),
            )
            bias_t = constp.tile([P, n_layers * F], fp32)
            nc.sync.dma_start(out=bias_t[:], in_=bias_d[:, :])
            wfc_t = constp.tile([F, 1], fp32)
            nc.sync.dma_start(out=wfc_t[:], in_=wfc_d[:, :])
            bfc_t = constp.tile([P, 1], fp32)
            nc.sync.dma_start(out=bfc_t[:], in_=bfc_d[:, :])
            ident_t = constp.tile([P, P], fp32)
            make_identity(nc, ident_t[:])
            zrow_t = constp.tile([2, F], fp32)
            nc.vector.memset(zrow_t[:], 0.0)

            T_bufs = [tsp.tile([P, NTF], fp32, tag=f"T{i}", name=f"Tbuf{i}")
                      for i in range(3)]
            S_t = wp.tile([P, NTF], fp32, tag="S")
            SB_t = wp.tile([P, NTF], fp32, tag="SB")

            nc.sync.dma_start(out=T_bufs[0][:], in_=t0shard_d[:, :])

            def matmul_terms(src_t, l, k, acc_sb):
                mm = ppa.tile([P, NTF], fp32, tag="mm")
                for t in range(NT):
                    tp = ppt.tile([F, P], fp32, tag="tp")
                    nc.tensor.transpose(
                        out=tp[:], in_=src_t[:, t * F:(t + 1) * F],
                        identity=ident_t[:],
                    )
                    lhsT = sp.tile([F, P], fp32, tag="lhsT")
                    nc.vector.tensor_copy(out=lhsT[:], in_=tp[:])
                    nc.tensor.matmul(
                        out=mm[:, t * F:(t + 1) * F],
                        lhsT=lhsT[:],
                        rhs=w_t[:, (l * S + k) * F:(l * S + k + 1) * F],
                        start=True,
                        stop=True,
                    )
                if k == 0:
                    nc.vector.tensor_copy(out=acc_sb[:], in_=mm[:])
                else:
                    nc.vector.tensor_tensor(
                        out=acc_sb[:], in0=acc_sb[:], in1=mm[:],
                        op=mybir.AluOpType.add,
                    )

            def spmv_gather_reduce(table_ap):
                tabA = table_ap[0:min(IDX_MAX, pl.table_rows), :]
                tabB = table_ap[pl.b_base:pl.table_rows, :]
                for gi, (t0, t1, WA, WB) in enumerate(pl.groups):
                    callA = pl.calls[2 * gi]
                    callB = pl.calls[2 * gi + 1]
                    g_t = gp.tile([P, (WA + WB) * F], fp32, tag="G")
                    for call, tab, woff, cum, DD, dst in (
                        (callA, tabA, 0, pl.cumA, pl.DA, S_t),
                        (callB, tabB, WA, pl.cumB, pl.DB, SB_t),
                    ):
                        # SWDGE carveout limit: <=1024 descriptors per call
                        for s0 in range(0, call["W"], 8):
                            w = min(8, call["W"] - s0)
                            nc.gpsimd.dma_gather(
                                g_t[:, (woff + s0) * F:(woff + s0 + w) * F].rearrange(
                                    "p (w f) -> p w f", f=F
                                ),
                                tab,
                                idx_t[:, call["col_off"] + 8 * s0:
                                      call["col_off"] + 8 * (s0 + w)],
                                w * P,
                                w * P,
                                F,
                            )
                        for t in range(t0, t1):
                            D = int(DD[t])
                            if D == 0:
                                nc.vector.memset(dst[:, t * F:(t + 1) * F], 0.0)
                                continue
                            off = woff + int(cum[t] - cum[t0])
                            gv = g_t[:, off * F:(off + D) * F].rearrange(
                                "p (d f) -> p f d", f=F
                            )
                            nc.vector.tensor_reduce(
                                out=dst[:, t * F:(t + 1) * F], in_=gv,
                                axis=mybir.AxisListType.X,
                                op=mybir.AluOpType.add,
                            )
                nc.vector.tensor_tensor(
                    out=S_t[:], in0=S_t[:], in1=SB_t[:], op=mybir.AluOpType.add
                )

            def stage_table(src_t):
                """table <- AllGather(dis * src)."""
                nc.vector.tensor_tensor(
                    out=SB_t[:], in0=src_t[:], in1=dis_t[:], op=mybir.AluOpType.mult
                )
                bounce = dp.tile([pl.shard_rows, F], fp32, tag="bounce")
                nc.sync.dma_start(
                    out=bounce[0:pl.slots, :].rearrange("(p r) f -> p r f", p=P),
                    in_=SB_t[:].rearrange("p (r f) -> p r f", f=F),
                )
                nc.sync.dma_start(out=bounce[pl.slots:pl.shard_rows, :], in_=zrow_t[:])
                table = dp.tile([pl.table_rows, F], fp32, tag="table")
                nc.gpsimd.collective_compute(
                    "AllGather",
                    mybir.AluOpType.bypass,
                    replica_groups=[list(range(pl.ncores))],
                    ins=[bounce[:, :].opt()],
                    outs=[table[:, :].opt()],
                )
                return table

            acc_sb = wp.tile([P, NTF], fp32, tag="accsb")
            cur = 0
            for l in range(n_layers):
                table = table0_d.ap() if l == 0 else stage_table(T_bufs[cur])
                matmul_terms(T_bufs[cur], l, 0, acc_sb)
                tm2 = tm1 = cur
                for k in range(1, S):
                    spmv_gather_reduce(table)
                    nc.vector.tensor_tensor(
                        out=S_t[:], in0=S_t[:], in1=av2_t[:], op=mybir.AluOpType.mult
                    )
                    if k == 1:
                        new = (cur + 1) % 3
                        nc.vector.tensor_scalar(
                            out=S_t[:], in0=S_t[:], scalar1=0.5, scalar2=None,
                            op0=mybir.AluOpType.mult,
                        )
                        nc.vector.tensor_scalar(
                            out=SB_t[:], in0=T_bufs[cur][:], scalar1=c1, scalar2=None,
                            op0=mybir.AluOpType.mult,
                        )
                        nc.vector.tensor_tensor(
                            out=T_bufs[new][:], in0=S_t[:], in1=SB_t[:],
                            op=mybir.AluOpType.add,
                        )
                        tm2, tm1 = cur, new
                    else:
                        new = 3 - tm1 - tm2
                        nc.vector.tensor_scalar(
                            out=SB_t[:], in0=T_bufs[tm1][:], scalar1=c2, scalar2=None,
                            op0=mybir.AluOpType.mult,
                        )
                        nc.vector.tensor_tensor(
                            out=S_t[:], in0=S_t[:], in1=SB_t[:],
                            op=mybir.AluOpType.add,
                        )
                        nc.vector.tensor_tensor(
                            out=T_bufs[new][:], in0=S_t[:], in1=T_bufs[tm2][:],
                            op=mybir.AluOpType.subtract,
                        )
                        tm2, tm1 = tm1, new
                    if k < S - 1:
                        table = stage_table(T_bufs[tm1])
                    matmul_terms(T_bufs[tm1], l, k, acc_sb)
                # layer output: relu(acc + bias) -> free T buffer
                outb = 3 - tm1 - tm2
                for t in range(NT):
                    nc.vector.tensor_tensor(
                        out=T_bufs[outb][:, t * F:(t + 1) * F],
                        in0=acc_sb[:, t * F:(t + 1) * F],
                        in1=bias_t[:, l * F:(l + 1) * F],
                        op=mybir.AluOpType.add,
                    )
                nc.vector.tensor_scalar(
                    out=T_bufs[outb][:], in0=T_bufs[outb][:], scalar1=0.0,
                    scalar2=None, op0=mybir.AluOpType.max,
                )
                cur = outb

            # ---- final FC ----
            out_sb = wp.tile([P, NT], fp32, tag="outsb")
            for t in range(NT):
                tp = ppt.tile([F, P], fp32, tag="tp")
                nc.tensor.transpose(
                    out=tp[:], in_=T_bufs[cur][:, t * F:(t + 1) * F],
                    identity=ident_t[:],
                )
                lhsT = sp.tile([F, P], fp32, tag="lhsT")
                nc.vector.tensor_copy(out=lhsT[:], in_=tp[:])
                fc_ps = ppt.tile([P, 1], fp32, tag="tp")
                nc.tensor.matmul(
                    out=fc_ps[:], lhsT=lhsT[:], rhs=wfc_t[:], start=True, stop=True
                )
                nc.vector.tensor_tensor(
                    out=out_sb[:, t:t + 1], in0=fc_ps[:], in1=bfc_t[:],
                    op=mybir.AluOpType.add,
                )
            nc.sync.dma_start(out=out_d[:, :], in_=out_sb[:])

    nc.compile()
    return nc


# ----------------------------------------------------------------------
# entry point
# ----------------------------------------------------------------------
def _prepare(inputs):
    x = np.asarray(inputs["x"], dtype=np.float32)
    edge_index = np.asarray(inputs["edge_index"])
    lam = float(np.asarray(inputs["lambda_max"]).reshape(-1)[0])
    n, f_in = x.shape
    row = edge_index[0].astype(np.int64)
    col = edge_index[1].astype(np.int64)

    pl = build_plan(row, col, n)
    deg = pl.deg
    dis = np.where(deg > 0, 1.0 / np.sqrt(np.maximum(deg, 1)), 0.0).astype(np.float32)

    x_pad = np.zeros((n, F), np.float32)
    x_pad[:, :f_in] = x
    W1 = np.asarray(inputs["W1"], dtype=np.float32)
    W1p = np.zeros((S, F, F), np.float32)
    W1p[:, :f_in, :] = W1
    wmat = np.stack([W1p,
                     np.asarray(inputs["W2"], np.float32),
                     np.asarray(inputs["W3"], np.float32)])
    biasb = np.zeros((P, 3 * F), np.float32)
    biasb[:, 0:F] = np.asarray(inputs["b1"], np.float32)
    biasb[:, F:2 * F] = np.asarray(inputs["b2"], np.float32)
    biasb[:, 2 * F:3 * F] = np.asarray(inputs["b3"], np.float32)
    bfc = float(np.asarray(inputs["bfc"]).reshape(-1)[0])

    NT, NTF = pl.ntiles, pl.ntiles * F
    a2 = -(4.0 / lam)
    table0 = np.zeros((pl.table_rows, F), np.float32)
    in_maps = []
    i = np.arange(pl.per_core)
    t_, p_ = i // P, i % P
    for c in range(pl.ncores):
        perm = pl.perms[c]
        sh = np.zeros((P, NT, F), np.float32)
        sh[p_, t_] = x_pad[perm]
        dl = np.zeros((P, NT), np.float32)
        dl[p_, t_] = dis[perm]
        table0[c * pl.shard_rows: c * pl.shard_rows + pl.slots] = (
            sh * dl[:, :, None]).reshape(P * NT, F)
        av2 = np.repeat((a2 * dl)[:, :, None], F, axis=2).reshape(P, NTF)
        disx = np.repeat(dl[:, :, None], F, axis=2).reshape(P, NTF)
        in_maps.append(dict(
            t0shard=np.ascontiguousarray(sh.reshape(P, NTF)),
            idx=np.ascontiguousarray(pl.idx_img[c]),
            av2exp=np.ascontiguousarray(av2.astype(np.float32)),
            disexp=np.ascontiguousarray(disx.astype(np.float32)),
            wmat=wmat,
            biasb=biasb,
            wfc=np.asarray(inputs["Wfc"], np.float32),
            bfc=np.full((P, 1), bfc, np.float32),
        ))
    for m in in_maps:
        m["table0"] = table0
    return pl, lam, in_maps


def _run(inputs, trace=False):
    from concourse.bass_utils import run_bass_kernel_spmd

    pl, lam, in_maps = _prepare(inputs)
    nc = build_bass(pl, lam)
    res = run_bass_kernel_spmd(
        nc, in_maps, core_ids=list(range(pl.ncores)), trace=trace
    )
    n = pl.n
    y = np.zeros((n, 1), np.float32)
    i = np.arange(pl.per_core)
    for c in range(pl.ncores):
        o = np.asarray(res.results[c]["out"])
        y[pl.perms[c], 0] = o[i % P, i // P]
    return y, res


def kernel(**inputs) -> np.ndarray:
    y, _ = _run(inputs, trace=False)
    return y

